# revision 1
# baseline (speedup 1.0000x reference)
"""Trainium2 Bass kernel for nn_CrossAttention (efficient-attention form).

Reference computation per batch b:
    K = softmax(x2, axis=-1)           # over D
    Q = softmax(x2, axis=1)            # over N
    out = ((x @ K.T) @ Q) @ W.T + b

Reassociated (matmuls are associative; both softmaxes share exp(x2)):
    E  = exp(x2)                       # one exp serves both softmaxes
    K  = E * (1/rowsum(E))             # per-row scale
    G  = K.T @ Q = (K.T @ E) * diag(1/colsum(E))
    out = x @ (G @ W.T) + b            # single [N,D]@[D,D] matmul on x

This turns 2*N^2*D MACs into ~2*N*D^2 (4x fewer at N=2048, D=512), and the
colsum is recovered for free: rows of K sum to 1, so rowsum(K.T @ E over d)
= colsum(E).  Batch dim B=8 is sharded across the 8 cores (data parallel).

Layout: rows are loaded in pairs per partition ([128, 2, 512] tiles) so each
partition's DRAM footprint is 4KB-contiguous (DMA concat threshold), the E/K
matmul chain runs in bf16 (fast weight load, 1 cyc/col), and the x/C chain in
float32r (TF32-like, full fp32 storage).
"""

import os
import sys

import numpy as np

if "/opt/trn_rl_repo" not in sys.path:
    sys.path.insert(0, "/opt/trn_rl_repo")

import concourse.bass as bass
import concourse.bass_utils as bass_utils
import concourse.mybir as mybir
import concourse.tile as tile
from concourse import bacc
from concourse.bass import ds, ts
from concourse.bass_utils import run_bass_kernel_spmd
from concourse.masks import make_identity

# Let walrus hoist/overlap LDWEIGHTS (its own default; bass pins it off).
# Harmless if the flag is absent from the argv.
if os.environ.get("KERNEL_LDW_OPT", "0") == "1":
    _orig_run_command = bass_utils.run_command

    def _patched_run_command(argv, **kwargs):
        argv = [a.replace("--enable-ldw-opt=false", "--enable-ldw-opt=true")
                if isinstance(a, str) else a for a in argv]
        return _orig_run_command(argv, **kwargs)

    bass_utils.run_command = _patched_run_command

B, N, D = 8, 2048, 512
P = 128
T = 2             # rows per partition per group
G = N // (P * T)  # 8 row groups
DC = D // P       # 4 column chunks of D
F32 = mybir.dt.float32
F32R = mybir.dt.float32r
BF16 = mybir.dt.bfloat16

EK_DT = BF16    # dtype of the E/K (softmax) matmul chain
XC_DT = F32R    # dtype of the x / C (output) matmul chain

_CACHE = {}


def _build_nc():
    nc = bacc.Bacc("TRN2", target_bir_lowering=False, debug=False)
    x_d = nc.declare_dram_parameter("x", [N, D], F32, isOutput=False)
    x2_d = nc.declare_dram_parameter("x2", [N, D], F32, isOutput=False)
    w_d = nc.declare_dram_parameter("W", [D, D], F32, isOutput=False)
    b_d = nc.declare_dram_parameter("b", [D], F32, isOutput=False)
    out_d = nc.declare_dram_parameter("out", [N, D], F32, isOutput=True)

    # row n = g*256 + p*2 + t  -> per-partition DRAM span is 2 rows = 4KB
    x_t = x_d[:].rearrange("(g p t) d -> g p t d", p=P, t=T)
    x2_t = x2_d[:].rearrange("(g p t) d -> g p t d", p=P, t=T)
    out_t = out_d[:].rearrange("(g p t) d -> g p t d", p=P, t=T)
    w_t = w_d[:].rearrange("(j p) d -> p j d", p=P)

    # W^T transposes happen at these (g, t) slots (late enough for the DMA,
    # early enough for the normalize phase)
    w_slots = {(3, 0): 0, (3, 1): 1, (4, 0): 2, (4, 1): 3}

    with tile.TileContext(nc) as tc:
        with (
            tc.tile_pool(name="big", bufs=1) as big,
            tc.tile_pool(name="stage", bufs=8) as stage,
            tc.tile_pool(name="stagex", bufs=4) as stagex,
            tc.tile_pool(name="small", bufs=1) as small,
            tc.tile_pool(name="stats", bufs=4) as stats,
            tc.tile_pool(name="outp", bufs=2) as outp,
            tc.tile_pool(name="psA", bufs=1, space="PSUM") as psA,
            tc.tile_pool(name="psT", bufs=2, space="PSUM") as psT,
            tc.tile_pool(name="psO", bufs=2, space="PSUM") as psO,
        ):
            # ---- persistent SBUF tensors
            e_all = big.tile([P, G, T, D], EK_DT, tag="e_all")   # exp(x2)
            k_all = big.tile([P, G, T, D], EK_DT, tag="k_all")   # K rows
            xt_all = big.tile([P, DC, N], XC_DT, tag="xt_all")   # x^T
            mt_all = big.tile([P, DC, D], XC_DT, tag="mt_all")   # (K^T E)^T
            wt_all = big.tile([P, DC, D], F32, tag="wt_all")     # W^T
            v_all = big.tile([P, DC, D], XC_DT, tag="v_all")     # diag(s) W^T
            c_all = big.tile([P, DC, D], XC_DT, tag="c_all")     # G @ W^T
            wn_all = big.tile([P, DC, D], F32, tag="wn_all")     # W natural
            ident = small.tile([P, P], F32, tag="ident")
            bias_bc = small.tile([P, D], F32, tag="bias_bc")

            make_identity(nc, ident)
            b_ap = b_d[:]
            nc.gpsimd.dma_start(
                out=bias_bc,
                in_=bass.AP(tensor=b_ap.tensor, offset=b_ap.offset,
                            ap=[[0, P]] + list(b_ap.ap)),
            )

            # input stream DMAs on the sync (HWDGE) queues; x2 first
            # (the E/K matmul chain consumes it), x afterwards (transposes
            # lag two groups behind)
            x2_tiles = []
            x_tiles = []
            for g in range(G):
                x2_s = stage.tile([P, T, D], F32, tag="x2_s")
                nc.sync.dma_start(out=x2_s, in_=x2_t[g])
                x2_tiles.append(x2_s)
                if g >= 1:
                    x_s = stagex.tile([P, T, D], F32, tag="x_s")
                    nc.sync.dma_start(out=x_s, in_=x_t[g - 1])
                    x_tiles.append(x_s)
            for g in range(G - 1, G):
                x_s = stagex.tile([P, T, D], F32, tag="x_s")
                nc.sync.dma_start(out=x_s, in_=x_t[g])
                x_tiles.append(x_s)
            nc.sync.dma_start(out=wn_all, in_=w_t)

            # psum accumulator for M''^T = (K^T E)^T: 4 chunks x [128, 512]
            ps_m = psA.tile([P, DC, D], F32, tag="ps_m")

            def transpose_group(gx):
                x_s = x_tiles[gx]
                for t in range(T):
                    pt = psT.tile([P, DC, P], F32, tag="pt")
                    for j in range(DC):
                        nc.tensor.transpose(
                            pt[:, j, :], x_s[:, t, ts(j, P)], ident)
                    col = gx * (P * T) + t * P
                    # late copies ride the scalar engine (idle after the
                    # exp stream); early ones stay on DVE
                    if gx >= 4:
                        nc.scalar.copy(xt_all[:, :, ds(col, P)], pt)
                    else:
                        nc.vector.tensor_copy(xt_all[:, :, ds(col, P)], pt)

            def transpose_w(jw):
                pw = psT.tile([P, DC, P], F32, tag="pt")
                for kw in range(DC):
                    nc.tensor.transpose(
                        pw[:, kw, :], wn_all[:, jw, ts(kw, P)], ident)
                nc.vector.tensor_copy(wt_all[:, :, ts(jw, P)], pw)

            LAG = 1
            for g in range(G):
                x2_s = x2_tiles[g]
                for t in range(T):
                    e_i = e_all[:, g, t, :]
                    rs = stats.tile([P, 1], F32, tag="rs")
                    nc.scalar.activation(
                        out=e_i, in_=x2_s[:, t, :],
                        func=mybir.ActivationFunctionType.Exp,
                        accum_out=rs,
                    )
                    rr = stats.tile([P, 1], F32, tag="rr")
                    nc.vector.reciprocal(out=rr, in_=rs)
                    # K = E * (1/rowsum) on DVE (ACT is busy with exp
                    # and the late x^T copies)
                    nc.vector.tensor_scalar_mul(
                        k_all[:, g, t, :], e_i.bitcast(BF16), rr)
                # M''^T[d',d] += sum_n E[n,d'] K[n,d]; 8 bf16 matmuls
                # back-to-back so FWL stays engaged
                for t in range(T):
                    e_i = e_all[:, g, t, :]
                    k_i = k_all[:, g, t, :]
                    for j in range(DC):
                        nc.tensor.matmul(
                            ps_m[:, j, :],
                            lhsT=e_i[ts(0, P), ts(j, P)],
                            rhs=k_i,
                            start=(g == 0 and t == 0),
                            stop=(g == G - 1 and t == T - 1),
                        )
                # lagged x^T transposes keep the first matmuls ungated by
                # the x input stream
                if g >= LAG:
                    transpose_group(g - LAG)
                if g == G - 2:
                    transpose_w(0)
                    transpose_w(1)
                if g == G - 1:
                    transpose_w(2)
                    transpose_w(3)
            for gx in range(G - LAG, G):
                transpose_group(gx)

            # ---- normalize: s = 1/colsum(E); colsum = rowsum of M''^T chunks
            for j in range(DC):
                cs = stats.tile([P, 1], F32, tag="cs")
                nc.vector.tensor_scalar(
                    out=mt_all[:, j, :], in0=ps_m[:, j, :],
                    scalar1=1.0, scalar2=0.0,
                    op0=mybir.AluOpType.mult,
                    op1=mybir.AluOpType.add,
                    accum_out=cs,
                )
                sj = stats.tile([P, 1], F32, tag="sj")
                nc.vector.reciprocal(out=sj, in_=cs)
                nc.vector.tensor_scalar_mul(v_all[:, j, :], wt_all[:, j, :], sj)

            # ---- C = M'' diag(s) W^T  ([D, D])
            for k in range(DC):
                pc = psO.tile([P, D], F32, tag="po")
                for j in range(DC):
                    nc.tensor.matmul(
                        pc,
                        lhsT=mt_all[:, j, ts(k, P)],
                        rhs=v_all[:, j, :],
                        start=(j == 0), stop=(j == DC - 1),
                    )
                nc.scalar.copy(c_all[:, k, :], pc)

            # ---- out = x @ C + b
            for g in range(G):
                og = outp.tile([P, T, D], F32, tag="og")
                for t in range(T):
                    po = psO.tile([P, D], F32, tag="po")
                    col = g * (P * T) + t * P
                    for j in range(DC):
                        nc.tensor.matmul(
                            po,
                            lhsT=xt_all[:, j, ds(col, P)],
                            rhs=c_all[:, j, :],
                            start=(j == 0), stop=(j == DC - 1),
                        )
                    nc.vector.tensor_add(og[:, t, :], po, bias_bc)
                    nc.scalar.dma_start(out=out_t[g][:, t, :], in_=og[:, t, :])

    nc.compile()
    return nc


def get_nc():
    if "nc" not in _CACHE:
        _CACHE["nc"] = _build_nc()
    return _CACHE["nc"]


def kernel(x, x2, W, b, _trace=False):
    nc = get_nc()
    in_maps = [
        {
            "x": np.ascontiguousarray(x[i], dtype=np.float32),
            "x2": np.ascontiguousarray(x2[i], dtype=np.float32),
            "W": np.ascontiguousarray(W, dtype=np.float32),
            "b": np.ascontiguousarray(b, dtype=np.float32),
        }
        for i in range(B)
    ]
    res = run_bass_kernel_spmd(nc, in_maps, list(range(B)), trace=_trace)
    out = np.stack([res.results[i]["out"] for i in range(B)], axis=0)
    if _trace:
        _CACHE["last_results"] = res
    return out

